# revision 1
# baseline (speedup 1.0000x reference)
"""FRAUDRE InterAgg (3-relation GNN message passing) on 8 TRN2 NeuronCores.

Full-input contract: kernel(**inputs) takes the unsharded numpy inputs and
returns the full [8192, 192] output. Internally the batch node dimension is
sharded 8 ways (1024 nodes per core); the 1M x 64 feature table and alpha are
replicated to every core.

Math notes (vs the jax reference):
  r_r        = relu(concat(self, mean_r))          # (n, 128) per relation r
  w          = softmax(alpha, axis=1)              # (128, 3), rows sum to 1
  aggregated = sum_r r_r * w[:, r]                 # (n, 128)
  out        = concat(self, aggregated)            # (n, 192)
Since relu(concat(self, mean_r))[:, :64] == relu(self) for every r and the
softmax weights sum to 1 across relations, aggregated[:, :64] == relu(self).
Only the second half needs the weighted sum:
  out = [ self | relu(self) | sum_r (w[64:128, r]/deg) * relu(sum_r) ]

Raw Bass (no Tile): this walrus build only allows 1 packed sync wait per
instruction, so all cross-engine deps are standalone wait_ge instructions.
Pipeline: Pool issues indirect gathers (double-buffered), DVE reduces +
applies relu/weights, SP streams the output tiles back to DRAM.
"""

import numpy as np


def _import_concourse():
    try:
        import concourse.bass  # noqa: F401
    except ImportError:
        import sys

        for p in ("/opt/trn_rl_repo", "/root/.axon_site/_ro/trn_rl_repo"):
            if p not in sys.path:
                sys.path.insert(0, p)
        import concourse.bass  # noqa: F401


N_CORES = 8
NUM_NODES = 1_000_000
EMBED = 64
N_BATCH = 8192
DEG = 32
PER_CORE = N_BATCH // N_CORES  # 1024
P = 128


def build_nc(num_nodes=NUM_NODES, embed=EMBED, per_core=PER_CORE, deg=DEG):
    """Build the per-core Bass program (SPMD: same program on all cores)."""
    _import_concourse()
    from contextlib import ExitStack

    import concourse.bass as bass
    import concourse.mybir as mybir

    f32 = mybir.dt.float32
    i32 = mybir.dt.int32
    Exp = mybir.ActivationFunctionType.Exp

    assert per_core % P == 0
    n_tiles = per_core // P

    nc = bass.Bass()
    feats = nc.dram_tensor("features", [num_nodes, embed], f32, kind="ExternalInput")
    alpha = nc.dram_tensor("alpha", [2 * embed, 3], f32, kind="ExternalInput")
    nodes = nc.dram_tensor("nodes", [per_core], i32, kind="ExternalInput")
    neighs = [
        nc.dram_tensor(f"neigh{r}", [per_core, deg], i32, kind="ExternalInput")
        for r in range(3)
    ]
    out = nc.dram_tensor("out", [per_core, 3 * embed], f32, kind="ExternalOutput")
    w_dram = nc.dram_tensor("w_scratch", [2 * embed, 3], f32)

    with ExitStack() as ctx:
        e = ctx.enter_context

        # SBUF tensors
        alpha_sb = e(nc.sbuf_tensor([2 * embed, 3], f32))
        w_e = e(nc.sbuf_tensor([2 * embed, 3], f32))
        w_s = e(nc.sbuf_tensor([2 * embed, 1], f32))
        w_rs = e(nc.sbuf_tensor([2 * embed, 1], f32))
        w_sb = e(nc.sbuf_tensor([2 * embed, 3], f32))
        wb_sb = e(nc.sbuf_tensor([P, 3 * embed], f32))
        nodes_all = e(nc.sbuf_tensor([P, n_tiles], i32))
        idx_all = [
            e(nc.sbuf_tensor(f"idx_all{r}", [P, n_tiles * deg], i32))
            for r in range(3)
        ]
        out_sb = [
            e(nc.sbuf_tensor(f"out_sb{i}", [P, 3 * embed], f32)) for i in range(2)
        ]
        ng = [
            [e(nc.sbuf_tensor(f"ng{r}_{i}", [P, deg * embed], f32)) for i in range(2)]
            for r in range(3)
        ]
        rl = e(nc.sbuf_tensor([P, embed], f32))
        tmp = e(nc.sbuf_tensor([P, embed], f32))

        # semaphores
        alpha_sem = e(nc.semaphore("alpha_sem"))
        idx_sem = e(nc.semaphore("idx_sem"))
        e_sem = e(nc.semaphore("e_sem"))
        v_sem = e(nc.semaphore("v_sem"))
        wd_sem = e(nc.semaphore("wd_sem"))
        wb_sem = e(nc.semaphore("wb_sem"))
        g_self = [e(nc.semaphore(f"g_self{i}")) for i in range(2)]
        g_sem = [
            [e(nc.semaphore(f"g_sem{r}_{i}")) for i in range(2)] for r in range(3)
        ]
        c_sem = [e(nc.semaphore(f"c_sem{r}")) for r in range(3)]
        dve_done = e(nc.semaphore("dve_done"))
        st_sem = [e(nc.semaphore(f"st_sem{i}")) for i in range(2)]

        block = e(nc.Block())

        @block.sync
        def _(sync):
            sync.dma_start(out=alpha_sb[:], in_=alpha[:, :]).then_inc(alpha_sem, 16)
            with nc.allow_non_contiguous_dma(reason="one-time 4KB index load"):
                sync.dma_start(
                    out=nodes_all[:], in_=nodes[:].rearrange("(t p) -> p t", p=P)
                ).then_inc(idx_sem, 16)
            for r in range(3):
                sync.dma_start(
                    out=idx_all[r][:].rearrange("p (t j) -> p t j", j=deg),
                    in_=neighs[r][:, :].rearrange("(t p) j -> p t j", p=P),
                ).then_inc(idx_sem, 16)
            sync.wait_ge(v_sem, 1)
            sync.dma_start(out=w_dram[:, :], in_=w_sb[:]).then_inc(wd_sem, 16)
            for t in range(n_tiles):
                sync.wait_ge(dve_done, t + 1)
                sync.dma_start(
                    out=out[t * P : (t + 1) * P, :], in_=out_sb[t % 2][:]
                ).then_inc(st_sem[t % 2], 16)

        @block.scalar
        def _(scalar):
            scalar.wait_ge(alpha_sem, 16)
            scalar.activation(w_e[:], alpha_sb[:], Exp).then_inc(e_sem, 1)

        @block.vector
        def _(vector):
            vector.wait_ge(e_sem, 1)
            vector.reduce_sum(w_s[:], w_e[:], axis=mybir.AxisListType.X)
            vector.drain()
            vector.reciprocal(w_rs[:], w_s[:])
            vector.drain()
            vector.tensor_mul(w_sb[:], w_e[:], w_rs[:].to_broadcast([2 * embed, 3]))
            vector.drain()
            # fold the 1/deg neighbor-mean scale into the weights (only the
            # second-half weights are ever used, so this is safe)
            vector.tensor_scalar_mul(w_sb[:], w_sb[:], 1.0 / deg).then_inc(v_sem, 1)
            vector.wait_ge(wb_sem, 16)
            for t in range(n_tiles):
                ob = out_sb[t % 2]
                vector.wait_ge(g_self[t % 2], 16 * (t // 2 + 1))
                vector.tensor_relu(ob[:, embed : 2 * embed], ob[:, 0:embed])
                acc = ob[:, 2 * embed : 3 * embed]
                for r in range(3):
                    b = ng[r][t % 2]
                    vector.wait_ge(g_sem[r][t % 2], 16 * deg * (t // 2 + 1))
                    width = deg * embed
                    first = True
                    while width > embed:
                        half = width // 2
                        if not first:
                            vector.drain()
                        vector.tensor_add(b[:, 0:half], b[:, 0:half], b[:, half:width])
                        width = half
                        first = False
                    vector.drain()
                    vector.tensor_relu(rl[:], b[:, 0:embed]).then_inc(c_sem[r], 1)
                    vector.drain()
                    wb_r = wb_sb[:, r : 3 * embed : 3]  # stride-3: w[embed+f, r]/deg
                    if r == 0:
                        vector.tensor_mul(acc, rl[:], wb_r)
                    elif r == 1:
                        vector.tensor_mul(tmp[:], rl[:], wb_r)
                        vector.drain()
                        vector.tensor_add(acc, acc, tmp[:])
                    else:
                        vector.tensor_mul(tmp[:], rl[:], wb_r)
                        vector.drain()
                        vector.tensor_add(acc, acc, tmp[:]).then_inc(dve_done, 1)

        @block.gpsimd
        def _(gpsimd):
            gpsimd.wait_ge(wd_sem, 16)
            # partition-broadcast of the weights: wb_sb[p, f*3+r] = w[64+f, r]
            gpsimd.dma_start(
                out=wb_sb[:],
                in_=w_dram[embed : 2 * embed, :]
                .rearrange("f r -> (f r)")[None, :]
                .partition_broadcast(P),
            ).then_inc(wb_sem, 16)
            gpsimd.wait_ge(idx_sem, 64)
            for t in range(n_tiles):
                if t >= 2:
                    gpsimd.wait_ge(st_sem[t % 2], 16 * (t // 2))
                gpsimd.indirect_dma_start(
                    out=out_sb[t % 2][:, 0:embed],
                    out_offset=None,
                    in_=feats[:],
                    in_offset=bass.IndirectOffsetOnAxis(
                        ap=nodes_all[:, t : t + 1], axis=0
                    ),
                ).then_inc(g_self[t % 2], 16)
                for r in range(3):
                    if t >= 2:
                        gpsimd.wait_ge(c_sem[r], t - 1)
                    # one index per partition per DMA (HW limit): column j of
                    # this tile's index block gathers the j-th neighbor row of
                    # all 128 nodes into columns [j*embed, (j+1)*embed)
                    for j in range(deg):
                        gpsimd.indirect_dma_start(
                            out=ng[r][t % 2][:, j * embed : (j + 1) * embed],
                            out_offset=None,
                            in_=feats[:],
                            in_offset=bass.IndirectOffsetOnAxis(
                                ap=idx_all[r][:, t * deg + j : t * deg + j + 1],
                                axis=0,
                            ),
                        ).then_inc(g_sem[r][t % 2], 16)

    return nc


_NC_CACHE = {}


def _get_nc():
    if "nc" not in _NC_CACHE:
        _NC_CACHE["nc"] = build_nc()
    return _NC_CACHE["nc"]


def _run(inputs, trace=False, trace_kwargs=None):
    _import_concourse()
    from concourse.bass_utils import run_bass_kernel_spmd

    features = np.ascontiguousarray(np.asarray(inputs["features"], dtype=np.float32))
    alpha = np.ascontiguousarray(np.asarray(inputs["alpha"], dtype=np.float32))
    nodes = np.asarray(inputs["nodes"]).astype(np.int32)
    nis = [np.asarray(inputs[f"neigh_idx{r + 1}"]).astype(np.int32) for r in range(3)]

    nc = _get_nc()
    in_maps = []
    for c in range(N_CORES):
        sl = slice(c * PER_CORE, (c + 1) * PER_CORE)
        m = {
            "features": features,
            "alpha": alpha,
            "nodes": np.ascontiguousarray(nodes[sl]),
        }
        for r in range(3):
            m[f"neigh{r}"] = np.ascontiguousarray(nis[r][sl])
        in_maps.append(m)

    kw = {}
    if trace:
        kw["trace"] = True
        if trace_kwargs:
            kw.update(trace_kwargs)
    res = run_bass_kernel_spmd(nc, in_maps, list(range(N_CORES)), **kw)
    out_full = np.concatenate([res.results[c]["out"] for c in range(N_CORES)], axis=0)
    return out_full, res


def kernel(**inputs) -> np.ndarray:
    out, _ = _run(inputs)
    return out

